# revision 81
# baseline (speedup 1.0000x reference)
"""Trainium2 Bass kernel for nn_DilatedSpatioTemporalGCN.

Sharding: time axis T=64 across 8 cores (8 timesteps each) for the
per-timestep adjacency + GCN; dilated temporal conv halos move via one
small AllGather per layer (with the X[-1] broadcast folded into the same
collective). Final attention computed on every core; core 7's output (the
only one with the real t=63 chain) is returned.

Only the dependency cone of the output is computed: layer-1 GCN runs at
local t in {1,3,5,7}, layer-2 at {3,7} plus the virtual timestep, and the
temporal convs only where their outputs feed later layers or the result.

Self-contained: needs numpy + the concourse tree at /opt/trn_rl_repo.
"""
import sys

for _p in ('/opt/trn_rl_repo', '/root/.axon_site/_ro/trn_rl_repo'):
    if _p not in sys.path:
        sys.path.insert(0, _p)

import ml_dtypes
import numpy as np

import concourse.bass as bass
import concourse.mybir as mybir
import concourse.tile as tile
from concourse.bass_utils import run_bass_kernel_spmd

# ---------------------------------------------------------------------------
# Workaround: this walrus build rejects >1 sem wait on the Tile tail drain
# (CTRL-class instruction). Split the drain's waits across single-wait NOPs.
import bass_rust
from concourse.vector_clock import ScopedClock


def _patched_drain_and_barrier(self, tick_clock, wait_clock):
    drain_inst = self.nc.sync.drain()
    wait_clock.add_sem_waits(
        drain_inst.ins, ScopedClock({None: tick_clock.global_clock})
    )
    si = drain_inst.ins.sync_info
    waits = list(si.on_wait)
    if len(waits) > 1:
        drain_inst.ins.sync_info = bass_rust.SyncInfo(
            on_wait=[waits[0]], on_update=list(si.on_update)
        )
        for w in waits[1:]:
            nop = self.nc.sync.nop(nofuse=True, hint="split_drain_wait")
            nop.ins.sync_info = bass_rust.SyncInfo(on_wait=[w], on_update=[])

    self.nc.all_engine_barrier()
    assert self.sems is not None
    popped = self.nc._tile_sem_poison_stack.pop()
    assert popped is self._sem_poison
    self.nc.clear_and_free_semaphores(list(self.sems.allocated().values()))


tile.TileContext._drain_and_barrier = _patched_drain_and_barrier


def _split_multi_waits(nc):
    """This walrus build accepts at most one sync wait per instruction;
    hoist extra waits onto same-engine NOPs inserted just before."""
    for f in nc.m.functions:
        for bb in f.blocks:
            out = []
            for ins in bb.instructions:
                si = ins.sync_info
                if (si is not None and len(si.on_wait) > 1
                        and ins.engine is not None):
                    waits = list(si.on_wait)
                    for idx, w in enumerate(waits[:-1]):
                        nop = mybir.InstNoOp(name=f"{ins.name}_w{idx}",
                                             engine=ins.engine)
                        nop.sync_info = bass_rust.SyncInfo(
                            on_wait=[w], on_update=[])
                        out.append(nop)
                    ins.sync_info = bass_rust.SyncInfo(
                        on_wait=[waits[-1]], on_update=list(si.on_update))
                out.append(ins)
            bb.instructions = out
# ---------------------------------------------------------------------------

T, N, D = 64, 1000, 64
KSZ = 3
DILS = (1, 2, 4)
L = 3
NCORES = 8
TLOC = T // NCORES            # 8 local timesteps
NCH = 8                       # n-chunks
CH_SZ = [128] * 7 + [104]
CH_OFF = [128 * c for c in range(NCH)]
MH = [(0, 500), (500, 500)]   # m halves for contiguous SBUF operands
MHP = [(0, 500), (512, 500)]  # m halves for PSUM outputs (bank-aligned)

# timesteps whose GCN / conv outputs feed the final result (per layer)
G_T = [list(range(TLOC)), [1, 3, 5, 7], [3, 7]]   # gcn timesteps
C_T = [[1, 3, 5, 7], [3, 7], []]                  # conv timesteps (l2 special)
PAY = [[5, 6, 7], [3, 5, 7]]                      # AG halo payload (xg slices)
C7ROW = 7 * 3 * 64                                # core 7's slices in agout


def _ps3(ap):
    """[P, 1024] PSUM view -> [P, 2, 500] skipping the bank-pad gap."""
    return ap.rearrange("p (h q) -> p h q", h=2)[:, :, 0:500]


def _cs3(ap):
    """Contiguous [P, 1000] view -> [P, 2, 500] matching _ps3."""
    return ap.rearrange("p (h q) -> p h q", h=2)

f32 = mybir.dt.float32
f32r = mybir.dt.float32r
f16 = mybir.dt.float16
f8 = mybir.dt.float8e4
i32 = mybir.dt.int32
MTE_SCALE = 64.0   # host-side MTE scale to land values in fp8e4m3 range
AF = mybir.ActivationFunctionType
OP = mybir.AluOpType


def _prow(i):
    """Parity row-slice for local timestep i (for K-strips / conv outputs)."""
    return slice(0, 64) if i % 2 == 0 else slice(64, 128)


def _pcol(i):
    """x_sb/x_h pair-packed column block for local timestep i."""
    p = i // 2
    return slice(1000 * p, 1000 * p + 1000)


def _tcol(i):
    """Unpaired per-t column block (base-0 [64, 8000] tensors)."""
    return slice(1000 * i, 1000 * i + 1000)


def build_program(alpha: float):
    nc = bass.Bass()

    # ---- external I/O ----------------------------------------------------
    mte = nc.dram_tensor("mte", [TLOC, N, N], f8, kind="ExternalInput")
    x_loc = nc.dram_tensor("x_loc", [128, 4000], f32, kind="ExternalInput")
    x16_loc = nc.dram_tensor("x16_loc", [128, 4000], f16,
                             kind="ExternalInput")
    x63h_in = nc.dram_tensor("x63h_in", [64, N], f16, kind="ExternalInput")
    xprev_in = nc.dram_tensor("xprev_in", [64, N], f32, kind="ExternalInput")
    bdup = nc.dram_tensor("bdup", [128, 64], f16, kind="ExternalInput")
    gw_in = [nc.dram_tensor(f"gw{l}", [128, 64], f16, kind="ExternalInput")
             for l in range(L)]
    cw_in = [nc.dram_tensor(f"cw{l}", [64, 192], f16, kind="ExternalInput")
             for l in range(L)]
    gbias = nc.dram_tensor("gbias", [64, L], f32, kind="ExternalInput")
    cbias = nc.dram_tensor("cbias", [128, L], f32, kind="ExternalInput")
    onesh_in = nc.dram_tensor("onesh_in", [128, 512], f16, kind="ExternalInput")
    hoff_in = nc.dram_tensor("hoff_in", [64, 8], i32, kind="ExternalInput")
    hmask_in = nc.dram_tensor("hmask_in", [64, 1], f32, kind="ExternalInput")
    ident_in = nc.dram_tensor("ident_in", [128, 64], f32, kind="ExternalInput")
    attp = nc.dram_tensor("attp", [128, 4], f32, kind="ExternalInput")
    out_t = nc.dram_tensor("out", [N, D], f32, kind="ExternalOutput")

    beta = [1.0 - alpha, 1.0, 1.0]
    alph = [alpha, 0.0, 0.0]

    with tile.TileContext(nc) as tc:
        # ---- persistent SBUF tiles (one long-lived pool) ----------------
        perm = tc.alloc_tile_pool(name="perm", bufs=1)

        def ptile(shape, dtype, name):
            return perm.tile(shape, dtype, name=name, tag=name)

        x_sb = ptile([128, 4000], f32, "x_sb")      # pair-packed X (fp32)
        x_h = ptile([128, 4000], f16, "x_h")        # pair-packed X (fp16)
        # X[-1] ping-pong (A: layers 0/2, B: layer 1); rows 64:128 stay zero
        x63_h = ptile([128, N], f16, "x63_h")
        x63_b = ptile([128, N], f16, "x63_b")
        xprev = ptile([64, N], f32, "xprev")
        w_bdup = ptile([128, 64], f16, "w_bdup")
        w_gw = ptile([128, 64 * L], f16, "w_gw")
        w_cw = ptile([64, 192 * L], f16, "w_cw")
        w_gb = ptile([64, L], f32, "w_gb")
        w_cb = ptile([128, L], f32, "w_cb")
        onesh = ptile([128, 512], f16, "onesh")
        hoffs = ptile([64, 8], i32, "hoffs")
        hmask = ptile([64, 1], f32, "hmask")
        ident = ptile([128, 64], f32, "ident")
        attw = ptile([128, 4], f32, "attw")

        xg_h = ptile([64, TLOC * N], f16, "xg_h")       # per-t, base 0
        xw8 = ptile([128, TLOC * 512], f8, "xw8")       # fp8 XW (l0, x64)
        halo_a = ptile([64, 3 * N], f16, "halo_a")      # l0 halo
        halo_b = ptile([64, 3 * N], f16, "halo_b")      # l1 halo
        c7s = ptile([64, 3 * N], f16, "c7s")    # core 7's gathered xg tail
        xtb_h = ptile([128, TLOC * N], f16, "xtb_h")    # rows 64:128 zero
        xwones = ptile([128, TLOC * 1024], f16, "xwones")
        z_sb = ptile([64, TLOC * N], f16, "z_sb")       # per-t, base 0

        # p-chain (virtual timestep just before this core's range)
        x1p = ptile([64, N], f32, "x1p")
        x2p_h = ptile([64, N], f16, "x2p_h")
        xtbp_h = ptile([128, N], f16, "xtbp_h")
        xwonesp = ptile([128, 1024], f16, "xwonesp")
        zp = ptile([64, N], f16, "zp")
        xgp_h = ptile([64, N], f16, "xgp_h")

        res = [ptile([128, N], f32, f"res{l}") for l in range(L)]
        ODD = slice(64, 128)    # parity of local t = 7 (where res lives)

        # ---- DRAM tiles for collectives ---------------------------------
        with tc.tile_pool(name="dram", bufs=1, space="DRAM") as dram_pool:
            agin = [
                dram_pool.tile([3 * 64, N], f16, name=f"agin{l}")
                for l in range(2)
            ]
            agout = [
                dram_pool.tile([NCORES * 3 * 64, N], f16,
                               addr_space="Shared", name=f"agout{l}")
                for l in range(2)
            ]
            dgin = dram_pool.tile([64, 16], f16, name="dgin")
            dgout = dram_pool.tile([NCORES * 64, 16], f16,
                                   addr_space="Shared", name="dgout")

            # ---- load constants / inputs --------------------------------
            for p in range(4):
                nc.sync.dma_start(x_h[:, 1000 * p:1000 * p + 1000],
                                  x16_loc[:, 1000 * p:1000 * p + 1000])
            for p in range(4):
                nc.sync.dma_start(x_sb[:, 1000 * p:1000 * p + 1000],
                                  x_loc[:, 1000 * p:1000 * p + 1000])
            nc.sync.dma_start(x63_h[0:64, :], x63h_in[:])
            nc.sync.dma_start(xprev[:], xprev_in[:])
            nc.sync.dma_start(w_bdup[:], bdup[:])
            for l in range(L):
                nc.sync.dma_start(w_gw[:, 64 * l:64 * l + 64], gw_in[l][:])
                nc.sync.dma_start(w_cw[:, 192 * l:192 * l + 192], cw_in[l][:])
            nc.sync.dma_start(w_gb[:], gbias[:])
            nc.sync.dma_start(w_cb[:], cbias[:])
            nc.sync.dma_start(onesh[:], onesh_in[:])
            nc.sync.dma_start(hoffs[:], hoff_in[:])
            nc.sync.dma_start(hmask[:], hmask_in[:])
            nc.sync.dma_start(ident[:], ident_in[:])
            nc.sync.dma_start(attw[:], attp[:])

            # warm up the collective runtime early (overlaps layer-0 work)
            with tc.tile_pool(name="warm", bufs=1) as wpool:
                wtile = wpool.tile([64, 16], f16, name="wtile")
                nc.vector.memset(wtile[:], 0.0)
                nc.sync.dma_start(dgin[:], wtile[:])
                nc.gpsimd.collective_compute(
                    "AllGather", OP.bypass,
                    replica_groups=[list(range(NCORES))],
                    ins=[dgin[:]], outs=[dgout[:]],
                )

            # ---- helpers ------------------------------------------------
            def phase01_t(li, psA, i):
                """XtB / XW / z for one local timestep (x_h must be fresh)."""
                par = i % 2
                rs = _prow(i)
                tcol = _tcol(i)
                pco = 1000 * (i // 2)
                big = psA.tile([64, 1024], f32, name=f"xtbps{li}{i}",
                               tag="big", bufs=2)
                for (po, _), (mo, ms) in zip(MHP, MH):
                    nc.tensor.matmul(
                        big[:, po:po + ms],
                        w_bdup[rs, :],
                        x_h[rs, pco + mo:pco + mo + ms],
                        start=True, stop=True,
                        tile_position=(64 * par, 0),
                    )
                # PSUM drains on Scalar at l0 (it has slack there), on
                # Vector at l1/l2 where Scalar is exp-saturated
                if li == 0:
                    nc.scalar.activation(_cs3(xtb_h[0:64, tcol]),
                                         _ps3(big[:]), AF.Copy)
                else:
                    nc.vector.tensor_copy(_cs3(xtb_h[0:64, tcol]),
                                          _ps3(big[:]))

                bigT = psA.tile([64, 1024], f32, name=f"xwtps{li}{i}",
                                tag="big", bufs=2)
                for (po, _), (mo, ms) in zip(MHP, MH):
                    nc.tensor.matmul(
                        bigT[:, po:po + ms],
                        w_gw[rs, 64 * li:64 * li + 64],
                        x_h[rs, pco + mo:pco + mo + ms],
                        start=True, stop=True,
                        tile_position=(64 * par, 0),
                    )
                if li == 0:
                    nc.scalar.activation(_cs3(z_sb[:, tcol]), _ps3(bigT[:]),
                                         AF.Identity,
                                         bias=w_gb[:, li:li + 1], scale=0.5)
                else:
                    nc.vector.tensor_scalar(
                        _cs3(z_sb[:, tcol]), _ps3(bigT[:]),
                        0.5, w_gb[:, li:li + 1], OP.mult, OP.add)

                xwp = psA.tile([128, 512], f32, name=f"xwps{li}{i}",
                               tag="xw", bufs=2)
                for c in range(NCH):
                    sz = CH_SZ[c]
                    co = CH_OFF[c]
                    nc.tensor.matmul(
                        xwp[0:sz, 64 * c:64 * c + 64],
                        x_h[rs, pco + co:pco + co + sz],
                        w_gw[rs, 64 * li:64 * li + 64],
                        start=True, stop=True,
                        tile_position=(64 * par, 0),
                    )
                nc.vector.tensor_copy(
                    xwones[:, 1024 * i:1024 * i + 1024]
                    .rearrange("p (c x) -> p c x", c=NCH)[:, :, 0:64],
                    xwp[:].rearrange("p (c x) -> p c x", c=NCH),
                )
                if li == 0:
                    # fp8 copy (scaled x64) for the DoubleRow static mix;
                    # the last chunk only has 104 real rows, so zero its
                    # tail first and drain just the live rows (partition
                    # bases must be 32-aligned, so zero 64:128 then
                    # overwrite 0:104)
                    nc.vector.memset(
                        xw8[64:128, 512 * i + 448:512 * i + 512], 0.0)
                    nc.vector.tensor_scalar_mul(
                        xw8[:, 512 * i:512 * i + 448],
                        xwp[:, 0:448], MTE_SCALE)
                    nc.vector.tensor_scalar_mul(
                        xw8[0:104, 512 * i + 448:512 * i + 512],
                        xwp[0:104, 448:512], MTE_SCALE)

            def phase01_p(li, psA):
                """L2 p-chain phase 0/1 from x2p_h (base 0 everywhere)."""
                bigp = psA.tile([64, 1024], f32, name="xtbpp",
                                tag="big", bufs=2)
                for (po, _), (mo, ms) in zip(MHP, MH):
                    nc.tensor.matmul(
                        bigp[:, po:po + ms], w_bdup[0:64, :],
                        x2p_h[:, mo:mo + ms], start=True, stop=True)
                nc.scalar.activation(_cs3(xtbp_h[0:64, :]), _ps3(bigp[:]),
                                     AF.Copy)
                bigp2 = psA.tile([64, 1024], f32, name="xwtpp",
                                 tag="big", bufs=2)
                for (po, _), (mo, ms) in zip(MHP, MH):
                    nc.tensor.matmul(
                        bigp2[:, po:po + ms],
                        w_gw[0:64, 64 * li:64 * li + 64],
                        x2p_h[:, mo:mo + ms], start=True, stop=True)
                nc.scalar.activation(_cs3(zp[:]), _ps3(bigp2[:]),
                                     AF.Identity, bias=w_gb[:, li:li + 1],
                                     scale=0.5)
                xwpp = psA.tile([128, 512], f32, name="xwpsp",
                                tag="xw", bufs=2)
                for c in range(NCH):
                    sz = CH_SZ[c]
                    co = CH_OFF[c]
                    nc.tensor.matmul(
                        xwpp[0:sz, 64 * c:64 * c + 64],
                        x2p_h[:, co:co + sz],
                        w_gw[0:64, 64 * li:64 * li + 64],
                        start=True, stop=True)
                nc.vector.tensor_copy(
                    xwonesp[:].rearrange("p (c x) -> p c x", c=NCH)
                    [:, :, 0:64],
                    xwpp[:].rearrange("p (c x) -> p c x", c=NCH))

            def gcn_t(li, tag, tidx, x63_ap, xtb_ap, xwo, z_ap, xg_dst,
                      psSC, psGE, psGM, eP, wkP, mteS, sc_bufs, depth,
                      pre=None, defer_combine=False):
                """Full GCN for one timestep (all tiles at base 0):
                scores -> E -> message/denominator -> combine -> xg.
                The chunk loop is software-pipelined by `depth`: the PE
                runs scores(c+depth) before message(c) so the exp/max
                chain of chunk c hides behind later scores matmuls.
                The v-broadcast (and conv@7's psum) alias the gm banks at
                l0 and the sc rotation elsewhere, freeing PSUM room."""
                bl = beta[li]
                al = alph[li]
                ge = psGE.tile([128, 1024], f32, name=f"ge_{tag}", tag="ge")
                gm = None
                if li == 0:
                    gm = psGM.tile([128, 1024], f32, name=f"gm_{tag}",
                                   tag="gm")
                e_cs = {}
                mts = {}

                def mt_fetch(p):
                    """MTE chunk-pair prefetch into a [128, 2, 1024] tile
                    (DoubleRow rhs layout; the Ko step must be 16B-aligned,
                    hence the 1024 pad). Alternate issue queues so the
                    Sync sequencer isn't the single DMA-issue bottleneck.
                    Deep buffering rides out the collective's DMA bursts."""
                    mt = mteS.tile([128, 2, 1024], f8, name=f"mt_{tag}_{p}",
                                   tag="mt", bufs=3)
                    for ko in range(2):
                        c = 2 * p + ko
                        eng = nc.sync if ko == 0 else nc.scalar
                        eng.dma_start(
                            mt[0:CH_SZ[c], ko, 0:N],
                            mte[tidx, CH_OFF[c]:CH_OFF[c] + CH_SZ[c], :])
                    mts[p] = mt

                def chunk_front(c):
                    sz = CH_SZ[c]
                    co = CH_OFF[c]
                    sc = psSC.tile([128, 1024], f32, name=f"sc_{tag}_{c}",
                                   tag="sc", bufs=sc_bufs)
                    for (po, _), (mo, ms) in zip(MHP, MH):
                        nc.tensor.matmul(
                            sc[0:sz, po:po + ms],
                            xtb_ap[:, co:co + sz],
                            x63_ap[:, mo:mo + ms],
                            start=True, stop=True,
                        )
                    e_c = eP.tile([128, N], f16, name=f"e_{tag}_{c}",
                                  tag="e", bufs=4)
                    nc.scalar.activation(_cs3(e_c[0:sz, :]),
                                         _ps3(sc[0:sz, :]), AF.Exp)
                    nc.vector.tensor_scalar_max(e_c[0:sz, :], e_c[0:sz, :],
                                                1.0)
                    e_cs[c] = e_c
                    if li == 0 and c % 2 == 0 and (c + 4) // 2 < NCH // 2:
                        mt_fetch((c + 4) // 2)

                def chunk_back(c):
                    sz = CH_SZ[c]
                    e_c = e_cs[c]
                    for (po, _), (mo, ms) in zip(MHP, MH):
                        nc.tensor.matmul(
                            ge[:, po:po + ms],
                            xwo[0:sz, 128 * c:128 * c + 128],
                            e_c[0:sz, mo:mo + ms],
                            start=(c == 0), stop=(c == NCH - 1),
                        )
                    if li == 0 and c % 2 == 1:
                        # static mix in fp8 DoubleRow: one matmul eats a
                        # chunk PAIR (K=256 packed 2/cell)
                        p = c // 2
                        mt = mts[p]
                        xw3 = xw8[:, 512 * tidx:512 * tidx + 512].rearrange(
                            "q (c d) -> q c d", c=NCH)
                        for (po, _), (mo, ms) in zip(MHP, MH):
                            nc.tensor.matmul(
                                gm[0:64, po:po + ms],
                                xw3[:, 2 * p:2 * p + 2, :],
                                mt[:, :, mo:mo + ms],
                                start=(p == 0), stop=(p == NCH // 2 - 1),
                                perf_mode=mybir.MatmulPerfMode.DoubleRow,
                            )

                if li == 0:
                    for p in range(2):
                        mt_fetch(p)
                for c in range(depth):
                    chunk_front(c)
                if pre is not None:
                    # previous unit's combine: its v-broadcast matmul now
                    # sits behind this unit's first score matmuls on the
                    # PE queue, hiding the exp->vtmp latency
                    pre()
                for c in range(depth, NCH):
                    chunk_front(c)
                    chunk_back(c - depth)
                for c in range(NCH - depth, NCH):
                    chunk_back(c)

                # allocate the v-broadcast psum NOW so its version slot in
                # the pool rotation precedes the next unit's tiles even
                # when the combine is deferred past that unit's fronts
                vpool, vtag, vbufs = ((psGM, "gm", 1) if li == 0
                                      else (psSC, "sc", sc_bufs))
                vps = vpool.tile([128, 1024], f32, name=f"vp_{tag}",
                                 tag=vtag, bufs=vbufs)

                def combine():
                    # static-mix term: read gm BEFORE vps reuses its banks
                    g_t = None
                    if li == 0:
                        g_t = wkP.tile([64, N], f32, name=f"g_{tag}",
                                       tag="wkg", bufs=2)
                        nc.vector.tensor_scalar(
                            _cs3(g_t[:]), _ps3(gm[0:64, :]),
                            al / (2.0 * MTE_SCALE * MTE_SCALE),
                            None, OP.mult)
                    # v = 2 - D/N on rows 64:128 (fp16), then a K=1 matmul
                    # broadcasts row 64 down to partitions 0:64 in PSUM.
                    # On l1/l2 the Scalar engine is exp-saturated: Vector.
                    vtmp = wkP.tile([128, N], f16, name=f"vt_{tag}",
                                    tag="wkv", bufs=2)
                    if li == 0:
                        nc.scalar.activation(_cs3(vtmp[64:128, :]),
                                             _ps3(ge[64:128, :]), AF.Copy,
                                             bias=2.0, scale=-1.0 / N)
                    else:
                        nc.vector.tensor_scalar(_cs3(vtmp[64:128, :]),
                                                _ps3(ge[64:128, :]),
                                                -1.0 / N, 2.0,
                                                OP.mult, OP.add)
                    for (po, _), (mo, ms) in zip(MHP, MH):
                        nc.tensor.matmul(
                            vps[0:64, po:po + ms],
                            onesh[64:65, 0:64],
                            vtmp[64:65, mo:mo + ms],
                            start=True, stop=True,
                            tile_position=(64, 0),
                        )
                    u_t = wkP.tile([64, N], f32, name=f"u_{tag}", tag="wku",
                                   bufs=2)
                    nc.vector.tensor_scalar(_cs3(u_t[:]), _ps3(ge[0:64, :]),
                                            bl / (2.0 * N), None, OP.mult)
                    w_t = wkP.tile([64, N], f32, name=f"w_{tag}", tag="wkw",
                                   bufs=2)
                    nc.vector.tensor_tensor(_cs3(w_t[:]), _cs3(u_t[:]),
                                            _ps3(vps[0:64, :]), OP.mult)
                    if li == 0:
                        nc.vector.tensor_add(w_t[:], w_t[:], g_t[:])
                    q_t = wkP.tile([64, N], f32, name=f"q_{tag}", tag="wku",
                                   bufs=2)
                    nc.vector.tensor_add(q_t[:], w_t[:], z_ap)
                    nc.vector.tensor_scalar_max(xg_dst, q_t[:], 0.0)

                if defer_combine:
                    return combine
                combine()
                return None

            def conv_taps(li, i, pool, tag, bufs, use_virtual=False):
                """Accumulate the 3 causal taps for local timestep i into a
                fresh [64, N] psum region at i's parity base; negative t'
                reads halo slices (or xgp for L2's virtual tap)."""
                dil = DILS[li]
                par = i % 2
                halo = halo_a if li == 0 else halo_b
                y_ps = pool.tile([128, 1024], f32, name=f"y_{li}_{i}",
                                 tag=tag, bufs=bufs)
                orows = _prow(i)
                for k in range(KSZ):
                    tp = i - (KSZ - 1 - k) * dil
                    if tp >= 0:
                        rhs = xg_h[:, _tcol(tp)]
                    elif use_virtual and tp == -1:
                        rhs = xgp_h[:]
                    else:
                        slot = (5 + tp) // 2 if li == 1 else 3 + tp
                        assert 0 <= slot < 3, (li, i, tp)
                        rhs = halo[:, N * slot:N * slot + N]
                    for (po, _), (mo, ms) in zip(MHP, MH):
                        nc.tensor.matmul(
                            y_ps[orows, po:po + ms],
                            w_cw[:, 192 * li + 64 * k:192 * li + 64 * k + 64],
                            rhs[:, mo:mo + ms],
                            start=(k == 0), stop=(k == KSZ - 1),
                            tile_position=(0, 64 * par),
                        )
                return y_ps, orows

            def conv_finish(li, i, y_ps, orows, yP, save_res=False,
                            upd='both'):
                """bias+relu, optional res capture, X update. upd='xh'
                writes the next layer's fp16 X directly (skipping the
                fp32 x_sb update + recast) when no later conv at this t
                needs the fp32 residual."""
                y_t = yP.tile([128, N], f32, name=f"yt{li}{i}",
                              tag="yt", bufs=3)
                nc.scalar.activation(_cs3(y_t[orows, :]),
                                     _ps3(y_ps[orows, :]), AF.Relu,
                                     bias=w_cb[orows, li:li + 1])
                if save_res:
                    nc.vector.tensor_copy(res[li][ODD, :], y_t[ODD, :])
                dst = x_h if upd == 'xh' else x_sb
                nc.vector.tensor_tensor(
                    dst[orows, _pcol(i)], y_t[orows, :],
                    x_sb[orows, _pcol(i)], OP.add)

            def recv_halo_dmas(li, halo):
                """Arm the halo + core-7-tail pulls right after the AG
                trigger: they sit on the collective's semaphore (idle
                GpSimd queue) and fire the moment it lands."""
                for k in range(3):
                    nc.gpsimd.indirect_dma_start(
                        out=halo[:, N * k:N * k + N],
                        out_offset=None,
                        in_=agout[li][:],
                        in_offset=bass.IndirectOffsetOnAxis(
                            ap=hoffs[:, k:k + 1], axis=0),
                    )
                for k in range(3):
                    nc.gpsimd.dma_start(
                        c7s[:, N * k:N * k + N],
                        agout[li][C7ROW + 64 * k:C7ROW + 64 * k + 64, :])

            def mask_halo(halo):
                """Zero-edge mask; emit where the Vector queue is free to
                wait (right before the halo's consumers)."""
                nc.vector.tensor_scalar_mul(halo[:], halo[:], hmask[:])

            def x63_next(li, psP, xP, x63_cur, x63_dst):
                """Every core recomputes X[t=63] for the next layer from
                core 7's gathered xg tail (replaces shipping it): the
                taps at global t=63 are exactly the payload slices."""
                yp = psP.tile([128, 1024], f32, name=f"x63y{li}",
                              tag="gm" if li == 0 else "sc",
                              bufs=1 if li == 0 else 3)
                for k in range(KSZ):
                    for (po, _), (mo, ms) in zip(MHP, MH):
                        nc.tensor.matmul(
                            yp[0:64, po:po + ms],
                            w_cw[:, 192 * li + 64 * k:192 * li + 64 * k + 64],
                            c7s[:, N * k + mo:N * k + mo + ms],
                            start=(k == 0), stop=(k == KSZ - 1))
                y_t = xP.tile([64, N], f32, name=f"x63t{li}", tag="wku",
                              bufs=2)
                nc.scalar.activation(_cs3(y_t[:]), _ps3(yp[0:64, :]),
                                     AF.Relu, bias=w_cb[0:64, li:li + 1])
                nc.vector.tensor_tensor(x63_dst[0:64, :], y_t[:],
                                        x63_cur[0:64, :], OP.add)

            def pchain_conv(li, yP, psY):
                """Conv at the virtual timestep from halo slices; update
                the p-chain state (x1p after l0, x2p_h after l1)."""
                halo = halo_a if li == 0 else halo_b
                yp_ps = psY.tile([128, 1024], f32, name=f"y{li}p",
                                 tag="y", bufs=2)
                for k in range(KSZ):
                    slot = k
                    for (po, _), (mo, ms) in zip(MHP, MH):
                        nc.tensor.matmul(
                            yp_ps[0:64, po:po + ms],
                            w_cw[:, 192 * li + 64 * k:192 * li + 64 * k + 64],
                            halo[:, N * slot + mo:N * slot + mo + ms],
                            start=(k == 0), stop=(k == KSZ - 1))
                ypt = yP.tile([128, N], f32, name=f"y{li}pt",
                              tag="yt", bufs=3)
                nc.scalar.activation(_cs3(ypt[0:64, :]),
                                     _ps3(yp_ps[0:64, :]), AF.Relu,
                                     bias=w_cb[0:64, li:li + 1])
                if li == 0:
                    nc.vector.tensor_add(x1p[:], ypt[0:64, :], xprev[:])
                else:
                    x2p = yP.tile([128, N], f32, name="x2p", tag="yt",
                                  bufs=3)
                    nc.vector.tensor_add(x2p[0:64, :], ypt[0:64, :], x1p[:])
                    nc.vector.tensor_copy(x2p_h[:], x2p[0:64, :])

            def recast_pair(p):
                nc.vector.tensor_copy(x_h[:, 1000 * p:1000 * p + 1000],
                                      x_sb[:, 1000 * p:1000 * p + 1000])

            # ========================= schedule ==========================
            # ---- S1: l0 phase01 (all t); x_h comes fp16 from the host ---
            with tc.tile_pool(name="psA0", bufs=1, space="PSUM") as psA:
                for i in range(TLOC):
                    phase01_t(0, psA, i)

            # deferred startup fills (off the critical first-matmul path):
            # zero-pad the score contraction to k=128 so those matmuls
            # count as full-array activity (k=64 never reaches warm
            # clock), and write the constant "ones" halves of xwones
            nc.vector.memset(x63_h[64:128, :], 0.0)
            nc.vector.memset(x63_b[64:128, :], 0.0)
            nc.vector.memset(xtb_h[64:128, :], 0.0)
            nc.vector.memset(xtbp_h[64:128, :], 0.0)
            for i in range(TLOC):
                nc.vector.tensor_copy(
                    xwones[:, 1024 * i:1024 * i + 1024]
                    .rearrange("p (c x) -> p c x", c=NCH)[:, :, 64:128],
                    onesh[:].rearrange("p (c x) -> p c x", c=NCH),
                )
            nc.vector.tensor_copy(
                xwonesp[:].rearrange("p (c x) -> p c x", c=NCH)[:, :, 64:128],
                onesh[:].rearrange("p (c x) -> p c x", c=NCH),
            )

            # ---- S2/S3: l0 GCN (tail first), conv@7, early AG -----------
            with (
                tc.tile_pool(name="psSC0", bufs=1, space="PSUM") as psSC,
                tc.tile_pool(name="psGE0", bufs=1, space="PSUM") as psGE,
                tc.tile_pool(name="psGM0", bufs=1, space="PSUM") as psGM,
                tc.tile_pool(name="eP0", bufs=1) as eP,
                tc.tile_pool(name="wkP0", bufs=1) as wkP,
                tc.tile_pool(name="mteS0", bufs=1) as mteS,
                tc.tile_pool(name="yPa0", bufs=1) as yPa,
            ):
                def l0gcn(i, pre=None, defer=False):
                    return gcn_t(0, f"l0t{i}", i, x63_h[:],
                                 xtb_h[:, _tcol(i)],
                                 xwones[:, 1024 * i:1024 * i + 1024],
                                 z_sb[:, _tcol(i)], xg_h[:, _tcol(i)],
                                 psSC, psGE, psGM, eP, wkP, mteS, 2, 1,
                                 pre=pre, defer_combine=defer)

                for k, i in enumerate(PAY[0]):
                    l0gcn(i)
                    # payload DMA on the GpSimd queue: it precedes the AG
                    # trigger there anyway, and keeps the Sync queue free
                    # for the MTE prefetch stream.
                    nc.gpsimd.dma_start(agin[0][k * 64:(k + 1) * 64, :],
                                        xg_h[:, _tcol(i)])
                nc.gpsimd.collective_compute(
                    "AllGather", OP.bypass,
                    replica_groups=[list(range(NCORES))],
                    ins=[agin[0][:]], outs=[agout[0][:]],
                )
                recv_halo_dmas(0, halo_a)
                # conv@7 shares the gm PSUM banks (rows 64:128); it is off
                # the AG critical path now, filling the collective window
                y_ps, orows = conv_taps(0, 7, psGM, "gm", 1)
                conv_finish(0, 7, y_ps, orows, yPa, save_res=True)
                pend = None
                for i in (1, 2, 3, 4, 0):
                    pend = l0gcn(i, pre=pend, defer=True)
                pend()
                x63_next(0, psGM, wkP, x63_h, x63_b)
                mask_halo(halo_a)

            # ---- S4: l0 convs @3, @5 (no halo needed) -------------------
            with (
                tc.tile_pool(name="psY0a", bufs=1, space="PSUM") as psY,
                tc.tile_pool(name="yP0a", bufs=1) as yP,
            ):
                y_ps, orows = conv_taps(0, 3, psY, "y", 2)
                conv_finish(0, 3, y_ps, orows, yP)
                y_ps, orows = conv_taps(0, 5, psY, "y", 2)
                conv_finish(0, 5, y_ps, orows, yP, upd='xh')

            # ---- S5: l1 phase01 for t=3,5,7 -----------------------------
            with tc.tile_pool(name="psA1a", bufs=1, space="PSUM") as psA:
                recast_pair(1)
                phase01_t(1, psA, 3)
                phase01_t(1, psA, 5)
                recast_pair(3)
                phase01_t(1, psA, 7)

            # ---- S8: l1 GCN tail + early AG + conv@7 --------------------
            with (
                tc.tile_pool(name="psSC1", bufs=1, space="PSUM") as psSC,
                tc.tile_pool(name="psGE1", bufs=1, space="PSUM") as psGE,
                tc.tile_pool(name="eP1", bufs=1) as eP,
                tc.tile_pool(name="wkP1", bufs=1) as wkP,
                tc.tile_pool(name="yPa1", bufs=1) as yPa,
            ):
                def l1gcn(i, sc, ge, e, wk):
                    gcn_t(1, f"l1t{i}", i, x63_b[:], xtb_h[:, _tcol(i)],
                          xwones[:, 1024 * i:1024 * i + 1024],
                          z_sb[:, _tcol(i)], xg_h[:, _tcol(i)],
                          sc, ge, None, e, wk, None, 3, 2)

                for k, i in enumerate(PAY[1]):
                    l1gcn(i, psSC, psGE, eP, wkP)
                    nc.gpsimd.dma_start(agin[1][k * 64:(k + 1) * 64, :],
                                        xg_h[:, _tcol(i)])
                nc.gpsimd.collective_compute(
                    "AllGather", OP.bypass,
                    replica_groups=[list(range(NCORES))],
                    ins=[agin[1][:]], outs=[agout[1][:]],
                )
                recv_halo_dmas(1, halo_b)
                y_ps, orows = conv_taps(1, 7, psSC, "sc", 3)
                conv_finish(1, 7, y_ps, orows, yPa, save_res=True,
                            upd='xh')

            # ---- S6: deferred l0 conv@1 + p-chain: AG1-window work ------
            with (
                tc.tile_pool(name="psY0b", bufs=1, space="PSUM") as psY,
                tc.tile_pool(name="yP0b", bufs=1) as yP,
            ):
                y_ps, orows = conv_taps(0, 1, psY, "y", 2)
                conv_finish(0, 1, y_ps, orows, yP, upd='xh')
                pchain_conv(0, yP, psY)

            # ---- S7: l1 phase01@1 + l2 phase01@7 (AG1-window work) ------
            with tc.tile_pool(name="psA1b", bufs=1, space="PSUM") as psA:
                phase01_t(1, psA, 1)
                phase01_t(2, psA, 7)

            # ---- S9: l1 GCN @1 + next-layer X63 + l2 GCN @7 -------------
            with (
                tc.tile_pool(name="psSC1b", bufs=1, space="PSUM") as psSC,
                tc.tile_pool(name="psGE1b", bufs=1, space="PSUM") as psGE,
                tc.tile_pool(name="eP1b", bufs=1) as eP,
                tc.tile_pool(name="wkP1b", bufs=1) as wkP,
            ):
                l1gcn(1, psSC, psGE, eP, wkP)
                x63_next(1, psSC, wkP, x63_b, x63_h)
                # l2 gcn@7 needs only the new X63 + its phase01 (done in
                # S7): run it here so it overlaps S11/S12's serial chains
                gcn_t(2, "l2t7", 7, x63_h[:], xtb_h[:, _tcol(7)],
                      xwones[:, 1024 * 7:1024 * 7 + 1024],
                      z_sb[:, _tcol(7)], xg_h[:, _tcol(7)],
                      psSC, psGE, None, eP, wkP, None, 3, 2)

            # ---- S11: l1 conv@3 + p-chain (halo landed) -----------------
            with (
                tc.tile_pool(name="psY1b", bufs=1, space="PSUM") as psY,
                tc.tile_pool(name="yP1b", bufs=1) as yP,
            ):
                mask_halo(halo_b)
                y_ps, orows = conv_taps(1, 3, psY, "y", 2)
                conv_finish(1, 3, y_ps, orows, yP, upd='xh')
                pchain_conv(1, yP, psY)

            # ---- S12: l2 phase01 for t=3 + p-chain ----------------------
            with tc.tile_pool(name="psA2b", bufs=1, space="PSUM") as psA:
                phase01_t(2, psA, 3)
                phase01_p(2, psA)

            # ---- S13/S14: l2 GCN (t=3,7,p) + conv@7 (virtual) -----------
            with (
                tc.tile_pool(name="psSC2", bufs=1, space="PSUM") as psSC,
                tc.tile_pool(name="psGE2", bufs=1, space="PSUM") as psGE,
                tc.tile_pool(name="eP2", bufs=1) as eP,
                tc.tile_pool(name="wkP2", bufs=1) as wkP,
            ):
                pend = gcn_t(2, "l2t3", 3, x63_h[:], xtb_h[:, _tcol(3)],
                             xwones[:, 1024 * 3:1024 * 3 + 1024],
                             z_sb[:, _tcol(3)], xg_h[:, _tcol(3)],
                             psSC, psGE, None, eP, wkP, None, 3, 2,
                             defer_combine=True)
                pend = gcn_t(2, "l2tp", 0, x63_h[:], xtbp_h[:], xwonesp[:],
                             zp[:], xgp_h[:],
                             psSC, psGE, None, eP, wkP, None, 3, 2,
                             pre=pend, defer_combine=True)
                pend()
                y_ps, orows = conv_taps(2, TLOC - 1, psSC, "sc", 3,
                                        use_virtual=True)
                nc.scalar.activation(_cs3(res[2][orows, :]),
                                     _ps3(y_ps[orows, :]), AF.Relu,
                                     bias=w_cb[orows, 2:3])

            # ===================== attention (all cores) ==================
            with (
                tc.tile_pool(name="psAT", bufs=1, space="PSUM") as psAT,
                tc.tile_pool(name="atP", bufs=1) as atP,
            ):
                S_t = []
                for m in range(L):
                    sm = atP.tile([128, N], f32, name=f"S{m}", tag="S",
                                  bufs=3)
                    nc.scalar.activation(
                        sm[ODD, :], res[m][ODD, :], AF.Tanh,
                        bias=attw[ODD, 1:2], scale=attw[ODD, 0:1])
                    S_t.append(sm)
                for c in range(NCH):
                    sz = CH_SZ[c]
                    co = CH_OFF[c]
                    s_ps = psAT.tile([128, 4], f32, name=f"sps{c}",
                                     tag="sps", bufs=2)
                    for m in range(L):
                        nc.tensor.matmul(
                            s_ps[0:sz, m:m + 1], S_t[m][ODD, co:co + sz],
                            attw[ODD, 2:3], start=True, stop=True,
                            tile_position=(64, 0))
                    e_s = atP.tile([128, 4], f32, name=f"es{c}", tag="es",
                                   bufs=2)
                    nc.scalar.activation(e_s[0:sz, 0:3], s_ps[0:sz, 0:3],
                                         AF.Exp)
                    den = atP.tile([128, 1], f32, name=f"den{c}", tag="den",
                                   bufs=2)
                    nc.vector.tensor_reduce(
                        den[0:sz, :], e_s[0:sz, 0:3], mybir.AxisListType.X,
                        OP.add)
                    rden = atP.tile([128, 1], f32, name=f"rden{c}",
                                    tag="rden", bufs=2)
                    nc.vector.reciprocal(rden[0:sz, :], den[0:sz, :])
                    aw = atP.tile([128, 4], f32, name=f"aw{c}", tag="aw",
                                  bufs=2)
                    nc.vector.tensor_scalar_mul(
                        aw[0:sz, 0:3], e_s[0:sz, 0:3], rden[0:sz, :])
                    acc = atP.tile([128, 64], f32, name=f"acc{c}",
                                   tag="acc", bufs=2)
                    for m in range(L):
                        rt_ps = psAT.tile([128, 64], f32, name=f"rt{c}{m}",
                                          tag="rt", bufs=2)
                        nc.tensor.transpose(
                            rt_ps[0:sz, :], res[m][ODD, co:co + sz],
                            ident[ODD, :], tile_position=(64, 0))
                        if m == 0:
                            nc.vector.tensor_scalar_mul(
                                acc[0:sz, :], rt_ps[0:sz, :],
                                aw[0:sz, m:m + 1])
                        else:
                            tmp = atP.tile([128, 64], f32, name=f"tmp{c}{m}",
                                           tag="tmp", bufs=2)
                            nc.vector.tensor_scalar_mul(
                                tmp[0:sz, :], rt_ps[0:sz, :],
                                aw[0:sz, m:m + 1])
                            nc.vector.tensor_add(
                                acc[0:sz, :], acc[0:sz, :], tmp[0:sz, :])
                    nc.sync.dma_start(out_t[co:co + sz, :], acc[0:sz, :])

        perm.release()

    _split_multi_waits(nc)
    return nc


# ---------------------------------------------------------------------------
def _prep_inputs(inputs):
    """Build the 8 per-core input maps from the full problem inputs."""
    X = np.ascontiguousarray(np.asarray(inputs["node_embeddings"], np.float32))
    B = np.asarray(inputs["B_weight"], np.float32)
    MTE = np.asarray(inputs["static_MTE"], np.float32)
    gcn_W = np.asarray(inputs["gcn_W"], np.float32)
    gcn_b = np.asarray(inputs["gcn_b"], np.float32)
    conv_W = np.asarray(inputs["conv_W"], np.float32)
    conv_b = np.asarray(inputs["conv_b"], np.float32)
    W_a = np.asarray(inputs["W_a"], np.float32)
    v = np.asarray(inputs["v"], np.float32)
    b_a = np.asarray(inputs["b_a"], np.float32)

    def dup(a):  # [64, k] -> [128, k]
        return np.ascontiguousarray(np.concatenate([a, a], axis=0))

    bdup = dup(B.astype(np.float16))
    gws = [dup(gcn_W[l].astype(np.float16)) for l in range(L)]
    cws = []
    for l in range(L):
        blocks = [np.ascontiguousarray(conv_W[l, :, :, k].T).astype(np.float16)
                  for k in range(KSZ)]
        cws.append(np.ascontiguousarray(np.concatenate(blocks, axis=1)))
    gbias = np.ascontiguousarray(gcn_b.T).astype(np.float32)        # [64, 3]
    cbias = dup(np.ascontiguousarray(conv_b.T).astype(np.float32))  # [128, 3]
    onesh = np.ones((128, 512), np.float16)
    ident = dup(np.eye(64, dtype=np.float32))                       # [128, 64]
    attp = np.zeros((64, 4), np.float32)
    attp[:, 0] = np.diagonal(W_a)
    attp[:, 1] = b_a
    attp[:, 2] = v[:, 0]
    attp = dup(attp)                                                # [128, 4]

    x63h = np.ascontiguousarray(X[T - 1].T).astype(np.float16)      # [64, N]

    in_maps = []
    for j in range(NCORES):
        Xl = X[TLOC * j:TLOC * (j + 1)]              # [8, N, D]
        xl = np.zeros((128, 4000), np.float32)
        for i in range(TLOC):
            rs = slice(0, 64) if i % 2 == 0 else slice(64, 128)
            xl[rs, 1000 * (i // 2):1000 * (i // 2) + 1000] = Xl[i].T
        xprev = (np.ascontiguousarray(X[TLOC * j - 1].T) if j > 0
                 else np.zeros((D, N), np.float32)).astype(np.float32)
        src = (j - 1) % NCORES
        hoff = np.zeros((64, 8), np.int32)
        for k in range(3):
            hoff[:, k] = (src * 3 + k) * 64 + np.arange(64)
        hmask = np.full((64, 1), 0.0 if j == 0 else 1.0, np.float32)
        in_maps.append({
            "mte": np.ascontiguousarray(
                MTE[TLOC * j:TLOC * (j + 1)] * MTE_SCALE).astype(
                    ml_dtypes.float8_e4m3),
            "x_loc": xl,
            "x16_loc": xl.astype(np.float16),
            "x63h_in": x63h,
            "xprev_in": np.ascontiguousarray(xprev),
            "bdup": bdup,
            **{f"gw{l}": gws[l] for l in range(L)},
            **{f"cw{l}": cws[l] for l in range(L)},
            "gbias": gbias,
            "cbias": cbias,
            "onesh_in": onesh,
            "hoff_in": hoff,
            "hmask_in": hmask,
            "ident_in": ident,
            "attp": attp,
        })
    return in_maps


_prog_cache = {}


def run_kernel(inputs, trace=False):
    alpha = float(np.asarray(inputs["alpha"]))
    if alpha not in _prog_cache:
        _prog_cache[alpha] = build_program(alpha)
    nc = _prog_cache[alpha]
    in_maps = _prep_inputs(inputs)
    res = run_bass_kernel_spmd(nc, in_maps, list(range(NCORES)), trace=trace)
    out = np.asarray(res.results[7]["out"], np.float32)
    return out, res


def kernel(**inputs) -> np.ndarray:
    out, _ = run_kernel(inputs, trace=False)
    return out


# revision 84
# speedup vs baseline: 1.0666x; 1.0666x over previous
"""Trainium2 Bass kernel for nn_DilatedSpatioTemporalGCN.

Sharding: time axis T=64 across 8 cores (8 timesteps each) for the
per-timestep adjacency + GCN; dilated temporal conv halos move via one
small AllGather per layer (with the X[-1] broadcast folded into the same
collective). Final attention computed on every core; core 7's output (the
only one with the real t=63 chain) is returned.

Only the dependency cone of the output is computed: layer-1 GCN runs at
local t in {1,3,5,7}, layer-2 at {3,7} plus the virtual timestep, and the
temporal convs only where their outputs feed later layers or the result.

Self-contained: needs numpy + the concourse tree at /opt/trn_rl_repo.
"""
import sys

for _p in ('/opt/trn_rl_repo', '/root/.axon_site/_ro/trn_rl_repo'):
    if _p not in sys.path:
        sys.path.insert(0, _p)

import ml_dtypes
import numpy as np

import concourse.bass as bass
import concourse.mybir as mybir
import concourse.tile as tile
from concourse.bass_utils import run_bass_kernel_spmd

# ---------------------------------------------------------------------------
# Workaround: this walrus build rejects >1 sem wait on the Tile tail drain
# (CTRL-class instruction). Split the drain's waits across single-wait NOPs.
import bass_rust
from concourse.vector_clock import ScopedClock


def _patched_drain_and_barrier(self, tick_clock, wait_clock):
    drain_inst = self.nc.sync.drain()
    wait_clock.add_sem_waits(
        drain_inst.ins, ScopedClock({None: tick_clock.global_clock})
    )
    si = drain_inst.ins.sync_info
    waits = list(si.on_wait)
    if len(waits) > 1:
        drain_inst.ins.sync_info = bass_rust.SyncInfo(
            on_wait=[waits[0]], on_update=list(si.on_update)
        )
        for w in waits[1:]:
            nop = self.nc.sync.nop(nofuse=True, hint="split_drain_wait")
            nop.ins.sync_info = bass_rust.SyncInfo(on_wait=[w], on_update=[])

    self.nc.all_engine_barrier()
    assert self.sems is not None
    popped = self.nc._tile_sem_poison_stack.pop()
    assert popped is self._sem_poison
    self.nc.clear_and_free_semaphores(list(self.sems.allocated().values()))


tile.TileContext._drain_and_barrier = _patched_drain_and_barrier


def _split_multi_waits(nc):
    """This walrus build accepts at most one sync wait per instruction;
    hoist extra waits onto same-engine NOPs inserted just before."""
    for f in nc.m.functions:
        for bb in f.blocks:
            out = []
            for ins in bb.instructions:
                si = ins.sync_info
                if (si is not None and len(si.on_wait) > 1
                        and ins.engine is not None):
                    waits = list(si.on_wait)
                    for idx, w in enumerate(waits[:-1]):
                        nop = mybir.InstNoOp(name=f"{ins.name}_w{idx}",
                                             engine=ins.engine)
                        nop.sync_info = bass_rust.SyncInfo(
                            on_wait=[w], on_update=[])
                        out.append(nop)
                    ins.sync_info = bass_rust.SyncInfo(
                        on_wait=[waits[-1]], on_update=list(si.on_update))
                out.append(ins)
            bb.instructions = out
# ---------------------------------------------------------------------------

T, N, D = 64, 1000, 64
KSZ = 3
DILS = (1, 2, 4)
L = 3
NCORES = 8
TLOC = T // NCORES            # 8 local timesteps
NCH = 8                       # n-chunks
CH_SZ = [128] * 7 + [104]
CH_OFF = [128 * c for c in range(NCH)]
MH = [(0, 500), (500, 500)]   # m halves for contiguous SBUF operands
MHP = [(0, 500), (512, 500)]  # m halves for PSUM outputs (bank-aligned)

# timesteps whose GCN / conv outputs feed the final result (per layer)
G_T = [list(range(TLOC)), [1, 3, 5, 7], [3, 7]]   # gcn timesteps
C_T = [[1, 3, 5, 7], [3, 7], []]                  # conv timesteps (l2 special)
PAY = [[5, 6, 7], [3, 5, 7]]                      # AG halo payload (xg slices)
C7ROW = 7 * 3 * 64                                # core 7's slices in agout


def _ps3(ap):
    """[P, 1024] PSUM view -> [P, 2, 500] skipping the bank-pad gap."""
    return ap.rearrange("p (h q) -> p h q", h=2)[:, :, 0:500]


def _cs3(ap):
    """Contiguous [P, 1000] view -> [P, 2, 500] matching _ps3."""
    return ap.rearrange("p (h q) -> p h q", h=2)

f32 = mybir.dt.float32
f32r = mybir.dt.float32r
f16 = mybir.dt.float16
f8 = mybir.dt.float8e4
i32 = mybir.dt.int32
MTE_SCALE = 64.0   # host-side MTE scale to land values in fp8e4m3 range
AF = mybir.ActivationFunctionType
OP = mybir.AluOpType


def _prow(i):
    """Parity row-slice for local timestep i (for K-strips / conv outputs)."""
    return slice(0, 64) if i % 2 == 0 else slice(64, 128)


def _pcol(i):
    """x_sb/x_h pair-packed column block for local timestep i."""
    p = i // 2
    return slice(1000 * p, 1000 * p + 1000)


def _tcol(i):
    """Unpaired per-t column block (base-0 [64, 8000] tensors)."""
    return slice(1000 * i, 1000 * i + 1000)


def build_program(alpha: float):
    nc = bass.Bass()

    # ---- external I/O ----------------------------------------------------
    mte = nc.dram_tensor("mte", [TLOC, N, N], f8, kind="ExternalInput")
    x_loc = nc.dram_tensor("x_loc", [128, 4000], f32, kind="ExternalInput")
    x16_loc = nc.dram_tensor("x16_loc", [128, 4000], f16,
                             kind="ExternalInput")
    x63h_in = nc.dram_tensor("x63h_in", [64, N], f16, kind="ExternalInput")
    xprev_in = nc.dram_tensor("xprev_in", [64, N], f32, kind="ExternalInput")
    bdup = nc.dram_tensor("bdup", [128, 64], f16, kind="ExternalInput")
    gw_in = [nc.dram_tensor(f"gw{l}", [128, 64], f16, kind="ExternalInput")
             for l in range(L)]
    cw_in = [nc.dram_tensor(f"cw{l}", [64, 192], f16, kind="ExternalInput")
             for l in range(L)]
    gbias = nc.dram_tensor("gbias", [64, L], f32, kind="ExternalInput")
    cbias = nc.dram_tensor("cbias", [128, L], f32, kind="ExternalInput")
    onesh_in = nc.dram_tensor("onesh_in", [128, 512], f16, kind="ExternalInput")
    hoff_in = nc.dram_tensor("hoff_in", [64, 8], i32, kind="ExternalInput")
    hmask_in = nc.dram_tensor("hmask_in", [64, 1], f32, kind="ExternalInput")
    ident_in = nc.dram_tensor("ident_in", [128, 64], f32, kind="ExternalInput")
    attp = nc.dram_tensor("attp", [128, 4], f32, kind="ExternalInput")
    out_t = nc.dram_tensor("out", [N, D], f32, kind="ExternalOutput")

    beta = [1.0 - alpha, 1.0, 1.0]
    alph = [alpha, 0.0, 0.0]

    with tile.TileContext(nc) as tc:
        # ---- persistent SBUF tiles (one long-lived pool) ----------------
        perm = tc.alloc_tile_pool(name="perm", bufs=1)

        def ptile(shape, dtype, name):
            return perm.tile(shape, dtype, name=name, tag=name)

        x_sb = ptile([128, 4000], f32, "x_sb")      # pair-packed X (fp32)
        x_h = ptile([128, 4000], f16, "x_h")        # pair-packed X (fp16)
        # X[-1] ping-pong (A: layers 0/2, B: layer 1); rows 64:128 stay zero
        x63_h = ptile([128, N], f16, "x63_h")
        x63_b = ptile([128, N], f16, "x63_b")
        xprev = ptile([64, N], f32, "xprev")
        w_bdup = ptile([128, 64], f16, "w_bdup")
        w_gw = ptile([128, 64 * L], f16, "w_gw")
        w_cw = ptile([64, 192 * L], f16, "w_cw")
        w_gb = ptile([64, L], f32, "w_gb")
        w_cb = ptile([128, L], f32, "w_cb")
        onesh = ptile([128, 512], f16, "onesh")
        hoffs = ptile([64, 8], i32, "hoffs")
        hmask = ptile([64, 1], f32, "hmask")
        ident = ptile([128, 64], f32, "ident")
        attw = ptile([128, 4], f32, "attw")

        xg_h = ptile([64, TLOC * N], f16, "xg_h")       # per-t, base 0
        xw8 = ptile([128, TLOC * 512], f8, "xw8")       # fp8 XW (l0, x64)
        halo_a = ptile([64, 3 * N], f16, "halo_a")      # l0 halo
        halo_b = ptile([64, 3 * N], f16, "halo_b")      # l1 halo
        c7s = ptile([64, 3 * N], f16, "c7s")    # core 7's gathered xg tail
        xtb_h = ptile([128, TLOC * N], f16, "xtb_h")    # rows 64:128 zero
        xwones = ptile([128, TLOC * 1024], f16, "xwones")
        z_sb = ptile([64, TLOC * N], f16, "z_sb")       # per-t, base 0

        # p-chain (virtual timestep just before this core's range)
        x1p = ptile([64, N], f32, "x1p")
        x2p_h = ptile([64, N], f16, "x2p_h")
        xtbp_h = ptile([128, N], f16, "xtbp_h")
        xwonesp = ptile([128, 1024], f16, "xwonesp")
        zp = ptile([64, N], f16, "zp")
        xgp_h = ptile([64, N], f16, "xgp_h")

        res = [ptile([128, N], f32, f"res{l}") for l in range(L)]
        ODD = slice(64, 128)    # parity of local t = 7 (where res lives)

        # ---- DRAM tiles for collectives ---------------------------------
        with tc.tile_pool(name="dram", bufs=1, space="DRAM") as dram_pool:
            agin = [
                dram_pool.tile([3 * 64, N], f16, name=f"agin{l}")
                for l in range(2)
            ]
            agout = [
                dram_pool.tile([NCORES * 3 * 64, N], f16,
                               addr_space="Shared", name=f"agout{l}")
                for l in range(2)
            ]
            dgin = dram_pool.tile([64, 16], f16, name="dgin")
            dgout = dram_pool.tile([NCORES * 64, 16], f16,
                                   addr_space="Shared", name="dgout")

            # ---- load constants / inputs --------------------------------
            # tiny weight tensors first: the very first phase01 matmul
            # needs only x_h block 0 + w_bdup/w_gw/w_gb, so don't bury
            # them behind megabytes of X blocks in the DMA queue
            nc.sync.dma_start(x_h[:, 0:1000], x16_loc[:, 0:1000])
            nc.sync.dma_start(w_bdup[:], bdup[:])
            for l in range(L):
                nc.sync.dma_start(w_gw[:, 64 * l:64 * l + 64], gw_in[l][:])
            nc.sync.dma_start(w_gb[:], gbias[:])
            nc.sync.dma_start(onesh[:], onesh_in[:])
            for p in range(1, 4):
                nc.sync.dma_start(x_h[:, 1000 * p:1000 * p + 1000],
                                  x16_loc[:, 1000 * p:1000 * p + 1000])
            nc.sync.dma_start(x63_h[0:64, :], x63h_in[:])
            for l in range(L):
                nc.sync.dma_start(w_cw[:, 192 * l:192 * l + 192], cw_in[l][:])
            for p in range(4):
                nc.sync.dma_start(x_sb[:, 1000 * p:1000 * p + 1000],
                                  x_loc[:, 1000 * p:1000 * p + 1000])
            nc.sync.dma_start(xprev[:], xprev_in[:])
            nc.sync.dma_start(w_cb[:], cbias[:])
            nc.sync.dma_start(hoffs[:], hoff_in[:])
            nc.sync.dma_start(hmask[:], hmask_in[:])
            nc.sync.dma_start(ident[:], ident_in[:])
            nc.sync.dma_start(attw[:], attp[:])

            # warm up the collective runtime early (overlaps layer-0 work)
            with tc.tile_pool(name="warm", bufs=1) as wpool:
                wtile = wpool.tile([64, 16], f16, name="wtile")
                nc.vector.memset(wtile[:], 0.0)
                nc.sync.dma_start(dgin[:], wtile[:])
                nc.gpsimd.collective_compute(
                    "AllGather", OP.bypass,
                    replica_groups=[list(range(NCORES))],
                    ins=[dgin[:]], outs=[dgout[:]],
                )

            # ---- helpers ------------------------------------------------
            def phase01_t(li, psA, i):
                """XtB / XW / z for one local timestep (x_h must be fresh)."""
                par = i % 2
                rs = _prow(i)
                tcol = _tcol(i)
                pco = 1000 * (i // 2)
                big = psA.tile([64, 1024], f32, name=f"xtbps{li}{i}",
                               tag="big", bufs=2)
                for (po, _), (mo, ms) in zip(MHP, MH):
                    nc.tensor.matmul(
                        big[:, po:po + ms],
                        w_bdup[rs, :],
                        x_h[rs, pco + mo:pco + mo + ms],
                        start=True, stop=True,
                        tile_position=(64 * par, 0),
                    )
                # PSUM drains on Scalar at l0 (it has slack there), on
                # Vector at l1/l2 where Scalar is exp-saturated
                if li == 0:
                    nc.scalar.activation(_cs3(xtb_h[0:64, tcol]),
                                         _ps3(big[:]), AF.Copy)
                else:
                    nc.vector.tensor_copy(_cs3(xtb_h[0:64, tcol]),
                                          _ps3(big[:]))

                bigT = psA.tile([64, 1024], f32, name=f"xwtps{li}{i}",
                                tag="big", bufs=2)
                for (po, _), (mo, ms) in zip(MHP, MH):
                    nc.tensor.matmul(
                        bigT[:, po:po + ms],
                        w_gw[rs, 64 * li:64 * li + 64],
                        x_h[rs, pco + mo:pco + mo + ms],
                        start=True, stop=True,
                        tile_position=(64 * par, 0),
                    )
                if li == 0:
                    nc.scalar.activation(_cs3(z_sb[:, tcol]), _ps3(bigT[:]),
                                         AF.Identity,
                                         bias=w_gb[:, li:li + 1], scale=0.5)
                else:
                    nc.vector.tensor_scalar(
                        _cs3(z_sb[:, tcol]), _ps3(bigT[:]),
                        0.5, w_gb[:, li:li + 1], OP.mult, OP.add)

                xwp = psA.tile([128, 512], f32, name=f"xwps{li}{i}",
                               tag="xw", bufs=2)
                for c in range(NCH):
                    sz = CH_SZ[c]
                    co = CH_OFF[c]
                    nc.tensor.matmul(
                        xwp[0:sz, 64 * c:64 * c + 64],
                        x_h[rs, pco + co:pco + co + sz],
                        w_gw[rs, 64 * li:64 * li + 64],
                        start=True, stop=True,
                        tile_position=(64 * par, 0),
                    )
                nc.vector.tensor_copy(
                    xwones[:, 1024 * i:1024 * i + 1024]
                    .rearrange("p (c x) -> p c x", c=NCH)[:, :, 0:64],
                    xwp[:].rearrange("p (c x) -> p c x", c=NCH),
                )
                if li == 0:
                    # fp8 copy (scaled x64) for the DoubleRow static mix;
                    # the last chunk only has 104 real rows, so zero its
                    # tail first and drain just the live rows (partition
                    # bases must be 32-aligned, so zero 64:128 then
                    # overwrite 0:104)
                    nc.vector.memset(
                        xw8[64:128, 512 * i + 448:512 * i + 512], 0.0)
                    nc.vector.tensor_scalar_mul(
                        xw8[:, 512 * i:512 * i + 448],
                        xwp[:, 0:448], MTE_SCALE)
                    nc.vector.tensor_scalar_mul(
                        xw8[0:104, 512 * i + 448:512 * i + 512],
                        xwp[0:104, 448:512], MTE_SCALE)

            def phase01_p(li, psA):
                """L2 p-chain phase 0/1 from x2p_h (base 0 everywhere)."""
                bigp = psA.tile([64, 1024], f32, name="xtbpp",
                                tag="big", bufs=2)
                for (po, _), (mo, ms) in zip(MHP, MH):
                    nc.tensor.matmul(
                        bigp[:, po:po + ms], w_bdup[0:64, :],
                        x2p_h[:, mo:mo + ms], start=True, stop=True)
                nc.scalar.activation(_cs3(xtbp_h[0:64, :]), _ps3(bigp[:]),
                                     AF.Copy)
                bigp2 = psA.tile([64, 1024], f32, name="xwtpp",
                                 tag="big", bufs=2)
                for (po, _), (mo, ms) in zip(MHP, MH):
                    nc.tensor.matmul(
                        bigp2[:, po:po + ms],
                        w_gw[0:64, 64 * li:64 * li + 64],
                        x2p_h[:, mo:mo + ms], start=True, stop=True)
                nc.scalar.activation(_cs3(zp[:]), _ps3(bigp2[:]),
                                     AF.Identity, bias=w_gb[:, li:li + 1],
                                     scale=0.5)
                xwpp = psA.tile([128, 512], f32, name="xwpsp",
                                tag="xw", bufs=2)
                for c in range(NCH):
                    sz = CH_SZ[c]
                    co = CH_OFF[c]
                    nc.tensor.matmul(
                        xwpp[0:sz, 64 * c:64 * c + 64],
                        x2p_h[:, co:co + sz],
                        w_gw[0:64, 64 * li:64 * li + 64],
                        start=True, stop=True)
                nc.vector.tensor_copy(
                    xwonesp[:].rearrange("p (c x) -> p c x", c=NCH)
                    [:, :, 0:64],
                    xwpp[:].rearrange("p (c x) -> p c x", c=NCH))

            def gcn_t(li, tag, tidx, x63_ap, xtb_ap, xwo, z_ap, xg_dst,
                      psSC, psGE, psGM, eP, wkP, mteS, sc_bufs, depth,
                      pre=None, defer_combine=False):
                """Full GCN for one timestep (all tiles at base 0):
                scores -> E -> message/denominator -> combine -> xg.
                The chunk loop is software-pipelined by `depth`: the PE
                runs scores(c+depth) before message(c) so the exp/max
                chain of chunk c hides behind later scores matmuls.
                The v-broadcast (and conv@7's psum) alias the gm banks at
                l0 and the sc rotation elsewhere, freeing PSUM room."""
                bl = beta[li]
                al = alph[li]
                ge = psGE.tile([128, 1024], f32, name=f"ge_{tag}", tag="ge")
                gm = None
                if li == 0:
                    gm = psGM.tile([128, 1024], f32, name=f"gm_{tag}",
                                   tag="gm")
                e_cs = {}
                mts = {}

                def mt_fetch(p):
                    """MTE chunk-pair prefetch into a [128, 2, 1024] tile
                    (DoubleRow rhs layout; the Ko step must be 16B-aligned,
                    hence the 1024 pad). Alternate issue queues so the
                    Sync sequencer isn't the single DMA-issue bottleneck.
                    Deep buffering rides out the collective's DMA bursts."""
                    mt = mteS.tile([128, 2, 1024], f8, name=f"mt_{tag}_{p}",
                                   tag="mt", bufs=3)
                    for ko in range(2):
                        c = 2 * p + ko
                        eng = nc.sync if ko == 0 else nc.scalar
                        eng.dma_start(
                            mt[0:CH_SZ[c], ko, 0:N],
                            mte[tidx, CH_OFF[c]:CH_OFF[c] + CH_SZ[c], :])
                    mts[p] = mt

                def chunk_front(c):
                    sz = CH_SZ[c]
                    co = CH_OFF[c]
                    sc = psSC.tile([128, 1024], f32, name=f"sc_{tag}_{c}",
                                   tag="sc", bufs=sc_bufs)
                    for (po, _), (mo, ms) in zip(MHP, MH):
                        nc.tensor.matmul(
                            sc[0:sz, po:po + ms],
                            xtb_ap[:, co:co + sz],
                            x63_ap[:, mo:mo + ms],
                            start=True, stop=True,
                        )
                    e_c = eP.tile([128, N], f16, name=f"e_{tag}_{c}",
                                  tag="e", bufs=4)
                    nc.scalar.activation(_cs3(e_c[0:sz, :]),
                                         _ps3(sc[0:sz, :]), AF.Exp)
                    nc.vector.tensor_scalar_max(e_c[0:sz, :], e_c[0:sz, :],
                                                1.0)
                    e_cs[c] = e_c
                    if li == 0 and c % 2 == 0 and (c + 4) // 2 < NCH // 2:
                        mt_fetch((c + 4) // 2)

                def chunk_back(c):
                    sz = CH_SZ[c]
                    e_c = e_cs[c]
                    for (po, _), (mo, ms) in zip(MHP, MH):
                        nc.tensor.matmul(
                            ge[:, po:po + ms],
                            xwo[0:sz, 128 * c:128 * c + 128],
                            e_c[0:sz, mo:mo + ms],
                            start=(c == 0), stop=(c == NCH - 1),
                        )
                    if li == 0 and c % 2 == 1:
                        # static mix in fp8 DoubleRow: one matmul eats a
                        # chunk PAIR (K=256 packed 2/cell)
                        p = c // 2
                        mt = mts[p]
                        xw3 = xw8[:, 512 * tidx:512 * tidx + 512].rearrange(
                            "q (c d) -> q c d", c=NCH)
                        for (po, _), (mo, ms) in zip(MHP, MH):
                            nc.tensor.matmul(
                                gm[0:64, po:po + ms],
                                xw3[:, 2 * p:2 * p + 2, :],
                                mt[:, :, mo:mo + ms],
                                start=(p == 0), stop=(p == NCH // 2 - 1),
                                perf_mode=mybir.MatmulPerfMode.DoubleRow,
                            )

                if li == 0:
                    for p in range(2):
                        mt_fetch(p)
                for c in range(depth):
                    chunk_front(c)
                if pre is not None:
                    # previous unit's combine: its v-broadcast matmul now
                    # sits behind this unit's first score matmuls on the
                    # PE queue, hiding the exp->vtmp latency
                    pre()
                for c in range(depth, NCH):
                    chunk_front(c)
                    chunk_back(c - depth)
                for c in range(NCH - depth, NCH):
                    chunk_back(c)

                # allocate the v-broadcast psum NOW so its version slot in
                # the pool rotation precedes the next unit's tiles even
                # when the combine is deferred past that unit's fronts
                vpool, vtag, vbufs = ((psGM, "gm", 1) if li == 0
                                      else (psSC, "sc", sc_bufs))
                vps = vpool.tile([128, 1024], f32, name=f"vp_{tag}",
                                 tag=vtag, bufs=vbufs)

                def combine():
                    # static-mix term: read gm BEFORE vps reuses its banks
                    g_t = None
                    if li == 0:
                        g_t = wkP.tile([64, N], f32, name=f"g_{tag}",
                                       tag="wkg", bufs=2)
                        nc.vector.tensor_scalar(
                            _cs3(g_t[:]), _ps3(gm[0:64, :]),
                            al / (2.0 * MTE_SCALE * MTE_SCALE),
                            None, OP.mult)
                    # v = 2 - D/N on rows 64:128 (fp16), then a K=1 matmul
                    # broadcasts row 64 down to partitions 0:64 in PSUM.
                    # On l1/l2 the Scalar engine is exp-saturated: Vector.
                    vtmp = wkP.tile([128, N], f16, name=f"vt_{tag}",
                                    tag="wkv", bufs=2)
                    if li == 0:
                        nc.scalar.activation(_cs3(vtmp[64:128, :]),
                                             _ps3(ge[64:128, :]), AF.Copy,
                                             bias=2.0, scale=-1.0 / N)
                    else:
                        nc.vector.tensor_scalar(_cs3(vtmp[64:128, :]),
                                                _ps3(ge[64:128, :]),
                                                -1.0 / N, 2.0,
                                                OP.mult, OP.add)
                    for (po, _), (mo, ms) in zip(MHP, MH):
                        nc.tensor.matmul(
                            vps[0:64, po:po + ms],
                            onesh[64:65, 0:64],
                            vtmp[64:65, mo:mo + ms],
                            start=True, stop=True,
                            tile_position=(64, 0),
                        )
                    u_t = wkP.tile([64, N], f32, name=f"u_{tag}", tag="wku",
                                   bufs=2)
                    nc.vector.tensor_scalar(_cs3(u_t[:]), _ps3(ge[0:64, :]),
                                            bl / (2.0 * N), None, OP.mult)
                    w_t = wkP.tile([64, N], f32, name=f"w_{tag}", tag="wkw",
                                   bufs=2)
                    nc.vector.tensor_tensor(_cs3(w_t[:]), _cs3(u_t[:]),
                                            _ps3(vps[0:64, :]), OP.mult)
                    if li == 0:
                        nc.vector.tensor_add(w_t[:], w_t[:], g_t[:])
                    q_t = wkP.tile([64, N], f32, name=f"q_{tag}", tag="wku",
                                   bufs=2)
                    nc.vector.tensor_add(q_t[:], w_t[:], z_ap)
                    nc.vector.tensor_scalar_max(xg_dst, q_t[:], 0.0)

                if defer_combine:
                    return combine
                combine()
                return None

            def conv_taps(li, i, pool, tag, bufs, use_virtual=False):
                """Accumulate the 3 causal taps for local timestep i into a
                fresh [64, N] psum region at i's parity base; negative t'
                reads halo slices (or xgp for L2's virtual tap)."""
                dil = DILS[li]
                par = i % 2
                halo = halo_a if li == 0 else halo_b
                y_ps = pool.tile([128, 1024], f32, name=f"y_{li}_{i}",
                                 tag=tag, bufs=bufs)
                orows = _prow(i)
                for k in range(KSZ):
                    tp = i - (KSZ - 1 - k) * dil
                    if tp >= 0:
                        rhs = xg_h[:, _tcol(tp)]
                    elif use_virtual and tp == -1:
                        rhs = xgp_h[:]
                    else:
                        slot = (5 + tp) // 2 if li == 1 else 3 + tp
                        assert 0 <= slot < 3, (li, i, tp)
                        rhs = halo[:, N * slot:N * slot + N]
                    for (po, _), (mo, ms) in zip(MHP, MH):
                        nc.tensor.matmul(
                            y_ps[orows, po:po + ms],
                            w_cw[:, 192 * li + 64 * k:192 * li + 64 * k + 64],
                            rhs[:, mo:mo + ms],
                            start=(k == 0), stop=(k == KSZ - 1),
                            tile_position=(0, 64 * par),
                        )
                return y_ps, orows

            def conv_finish(li, i, y_ps, orows, yP, save_res=False,
                            upd='both'):
                """bias+relu, optional res capture, X update. upd='xh'
                writes the next layer's fp16 X directly (skipping the
                fp32 x_sb update + recast) when no later conv at this t
                needs the fp32 residual."""
                y_t = yP.tile([128, N], f32, name=f"yt{li}{i}",
                              tag="yt", bufs=3)
                nc.scalar.activation(_cs3(y_t[orows, :]),
                                     _ps3(y_ps[orows, :]), AF.Relu,
                                     bias=w_cb[orows, li:li + 1])
                if save_res:
                    nc.vector.tensor_copy(res[li][ODD, :], y_t[ODD, :])
                dst = x_h if upd == 'xh' else x_sb
                nc.vector.tensor_tensor(
                    dst[orows, _pcol(i)], y_t[orows, :],
                    x_sb[orows, _pcol(i)], OP.add)

            def recv_halo_dmas(li, halo):
                """Arm the halo + core-7-tail pulls right after the AG
                trigger: they sit on the collective's semaphore (idle
                GpSimd queue) and fire the moment it lands."""
                for k in range(3):
                    nc.gpsimd.indirect_dma_start(
                        out=halo[:, N * k:N * k + N],
                        out_offset=None,
                        in_=agout[li][:],
                        in_offset=bass.IndirectOffsetOnAxis(
                            ap=hoffs[:, k:k + 1], axis=0),
                    )
                for k in range(3):
                    nc.gpsimd.dma_start(
                        c7s[:, N * k:N * k + N],
                        agout[li][C7ROW + 64 * k:C7ROW + 64 * k + 64, :])

            def mask_halo(halo):
                """Zero-edge mask; emit where the Vector queue is free to
                wait (right before the halo's consumers)."""
                nc.vector.tensor_scalar_mul(halo[:], halo[:], hmask[:])

            def x63_next(li, psP, xP, x63_cur, x63_dst):
                """Every core recomputes X[t=63] for the next layer from
                core 7's gathered xg tail (replaces shipping it): the
                taps at global t=63 are exactly the payload slices."""
                yp = psP.tile([128, 1024], f32, name=f"x63y{li}",
                              tag="gm" if li == 0 else "sc",
                              bufs=1 if li == 0 else 3)
                for k in range(KSZ):
                    for (po, _), (mo, ms) in zip(MHP, MH):
                        nc.tensor.matmul(
                            yp[0:64, po:po + ms],
                            w_cw[:, 192 * li + 64 * k:192 * li + 64 * k + 64],
                            c7s[:, N * k + mo:N * k + mo + ms],
                            start=(k == 0), stop=(k == KSZ - 1))
                y_t = xP.tile([64, N], f32, name=f"x63t{li}", tag="wku",
                              bufs=2)
                nc.scalar.activation(_cs3(y_t[:]), _ps3(yp[0:64, :]),
                                     AF.Relu, bias=w_cb[0:64, li:li + 1])
                nc.vector.tensor_tensor(x63_dst[0:64, :], y_t[:],
                                        x63_cur[0:64, :], OP.add)

            def pchain_conv(li, yP, psY):
                """Conv at the virtual timestep from halo slices; update
                the p-chain state (x1p after l0, x2p_h after l1)."""
                halo = halo_a if li == 0 else halo_b
                yp_ps = psY.tile([128, 1024], f32, name=f"y{li}p",
                                 tag="y", bufs=2)
                for k in range(KSZ):
                    slot = k
                    for (po, _), (mo, ms) in zip(MHP, MH):
                        nc.tensor.matmul(
                            yp_ps[0:64, po:po + ms],
                            w_cw[:, 192 * li + 64 * k:192 * li + 64 * k + 64],
                            halo[:, N * slot + mo:N * slot + mo + ms],
                            start=(k == 0), stop=(k == KSZ - 1))
                ypt = yP.tile([128, N], f32, name=f"y{li}pt",
                              tag="yt", bufs=3)
                nc.scalar.activation(_cs3(ypt[0:64, :]),
                                     _ps3(yp_ps[0:64, :]), AF.Relu,
                                     bias=w_cb[0:64, li:li + 1])
                if li == 0:
                    nc.vector.tensor_add(x1p[:], ypt[0:64, :], xprev[:])
                else:
                    x2p = yP.tile([128, N], f32, name="x2p", tag="yt",
                                  bufs=3)
                    nc.vector.tensor_add(x2p[0:64, :], ypt[0:64, :], x1p[:])
                    nc.vector.tensor_copy(x2p_h[:], x2p[0:64, :])

            def recast_pair(p):
                nc.vector.tensor_copy(x_h[:, 1000 * p:1000 * p + 1000],
                                      x_sb[:, 1000 * p:1000 * p + 1000])

            # ========================= schedule ==========================
            # ---- S1: l0 phase01 (all t); x_h comes fp16 from the host ---
            with tc.tile_pool(name="psA0", bufs=1, space="PSUM") as psA:
                for i in range(TLOC):
                    phase01_t(0, psA, i)

            # deferred startup fills (off the critical first-matmul path):
            # zero-pad the score contraction to k=128 so those matmuls
            # count as full-array activity (k=64 never reaches warm
            # clock), and write the constant "ones" halves of xwones
            nc.vector.memset(x63_h[64:128, :], 0.0)
            nc.vector.memset(x63_b[64:128, :], 0.0)
            nc.vector.memset(xtb_h[64:128, :], 0.0)
            nc.vector.memset(xtbp_h[64:128, :], 0.0)
            for i in range(TLOC):
                nc.vector.tensor_copy(
                    xwones[:, 1024 * i:1024 * i + 1024]
                    .rearrange("p (c x) -> p c x", c=NCH)[:, :, 64:128],
                    onesh[:].rearrange("p (c x) -> p c x", c=NCH),
                )
            nc.vector.tensor_copy(
                xwonesp[:].rearrange("p (c x) -> p c x", c=NCH)[:, :, 64:128],
                onesh[:].rearrange("p (c x) -> p c x", c=NCH),
            )

            # ---- S2/S3: l0 GCN (tail first), conv@7, early AG -----------
            with (
                tc.tile_pool(name="psSC0", bufs=1, space="PSUM") as psSC,
                tc.tile_pool(name="psGE0", bufs=1, space="PSUM") as psGE,
                tc.tile_pool(name="psGM0", bufs=1, space="PSUM") as psGM,
                tc.tile_pool(name="eP0", bufs=1) as eP,
                tc.tile_pool(name="wkP0", bufs=1) as wkP,
                tc.tile_pool(name="mteS0", bufs=1) as mteS,
                tc.tile_pool(name="yPa0", bufs=1) as yPa,
            ):
                def l0gcn(i, pre=None, defer=False):
                    return gcn_t(0, f"l0t{i}", i, x63_h[:],
                                 xtb_h[:, _tcol(i)],
                                 xwones[:, 1024 * i:1024 * i + 1024],
                                 z_sb[:, _tcol(i)], xg_h[:, _tcol(i)],
                                 psSC, psGE, psGM, eP, wkP, mteS, 2, 1,
                                 pre=pre, defer_combine=defer)

                for k, i in enumerate(PAY[0]):
                    l0gcn(i)
                    # payload DMA on the GpSimd queue: it precedes the AG
                    # trigger there anyway, and keeps the Sync queue free
                    # for the MTE prefetch stream.
                    nc.gpsimd.dma_start(agin[0][k * 64:(k + 1) * 64, :],
                                        xg_h[:, _tcol(i)])
                nc.gpsimd.collective_compute(
                    "AllGather", OP.bypass,
                    replica_groups=[list(range(NCORES))],
                    ins=[agin[0][:]], outs=[agout[0][:]],
                )
                recv_halo_dmas(0, halo_a)
                # conv@7 shares the gm PSUM banks (rows 64:128); it is off
                # the AG critical path now, filling the collective window
                y_ps, orows = conv_taps(0, 7, psGM, "gm", 1)
                conv_finish(0, 7, y_ps, orows, yPa, save_res=True)
                pend = None
                for i in (1, 2, 3, 4):
                    pend = l0gcn(i, pre=pend, defer=True)
                pend()
                x63_next(0, psGM, wkP, x63_h, x63_b)
                mask_halo(halo_a)

            # ---- S4: l0 convs @3, @5 (no halo needed) -------------------
            with (
                tc.tile_pool(name="psY0a", bufs=1, space="PSUM") as psY,
                tc.tile_pool(name="yP0a", bufs=1) as yP,
            ):
                y_ps, orows = conv_taps(0, 3, psY, "y", 2)
                conv_finish(0, 3, y_ps, orows, yP)
                y_ps, orows = conv_taps(0, 5, psY, "y", 2)
                conv_finish(0, 5, y_ps, orows, yP, upd='xh')

            # ---- S5: l1 phase01 for t=3,5,7 -----------------------------
            with tc.tile_pool(name="psA1a", bufs=1, space="PSUM") as psA:
                recast_pair(1)
                phase01_t(1, psA, 3)
                phase01_t(1, psA, 5)
                recast_pair(3)
                phase01_t(1, psA, 7)

            # ---- S8: l1 GCN tail + early AG + conv@7 --------------------
            with (
                tc.tile_pool(name="psSC1", bufs=1, space="PSUM") as psSC,
                tc.tile_pool(name="psGE1", bufs=1, space="PSUM") as psGE,
                tc.tile_pool(name="eP1", bufs=1) as eP,
                tc.tile_pool(name="wkP1", bufs=1) as wkP,
                tc.tile_pool(name="yPa1", bufs=1) as yPa,
            ):
                def l1gcn(i, sc, ge, e, wk):
                    gcn_t(1, f"l1t{i}", i, x63_b[:], xtb_h[:, _tcol(i)],
                          xwones[:, 1024 * i:1024 * i + 1024],
                          z_sb[:, _tcol(i)], xg_h[:, _tcol(i)],
                          sc, ge, None, e, wk, None, 3, 2)

                for k, i in enumerate(PAY[1]):
                    l1gcn(i, psSC, psGE, eP, wkP)
                    nc.gpsimd.dma_start(agin[1][k * 64:(k + 1) * 64, :],
                                        xg_h[:, _tcol(i)])
                nc.gpsimd.collective_compute(
                    "AllGather", OP.bypass,
                    replica_groups=[list(range(NCORES))],
                    ins=[agin[1][:]], outs=[agout[1][:]],
                )
                recv_halo_dmas(1, halo_b)
                y_ps, orows = conv_taps(1, 7, psSC, "sc", 3)
                conv_finish(1, 7, y_ps, orows, yPa, save_res=True,
                            upd='xh')

            # ---- S9a: l0 gcn@0 — pure AG1-window work (it feeds only
            # conv@1 -> phase01@1 -> l1 gcn@1, all post-trigger) ---------
            with (
                tc.tile_pool(name="psSC0b", bufs=1, space="PSUM") as psSC0,
                tc.tile_pool(name="psGE0b", bufs=1, space="PSUM") as psGE0,
                tc.tile_pool(name="psGM0b", bufs=1, space="PSUM") as psGM0,
                tc.tile_pool(name="eP0b", bufs=1) as eP0,
                tc.tile_pool(name="wkP0b", bufs=1) as wkP0,
                tc.tile_pool(name="mteS0b", bufs=1) as mteS0,
            ):
                gcn_t(0, "l0t0", 0, x63_h[:], xtb_h[:, _tcol(0)],
                      xwones[:, 0:1024],
                      z_sb[:, _tcol(0)], xg_h[:, _tcol(0)],
                      psSC0, psGE0, psGM0, eP0, wkP0, mteS0, 2, 1)

            # ---- S6: deferred l0 conv@1 + p-chain: AG1-window work ------
            with (
                tc.tile_pool(name="psY0b", bufs=1, space="PSUM") as psY,
                tc.tile_pool(name="yP0b", bufs=1) as yP,
            ):
                y_ps, orows = conv_taps(0, 1, psY, "y", 2)
                conv_finish(0, 1, y_ps, orows, yP, upd='xh')
                pchain_conv(0, yP, psY)

            # ---- S7: l1 phase01@1 + l2 phase01@7 (AG1-window work) ------
            with tc.tile_pool(name="psA1b", bufs=1, space="PSUM") as psA:
                phase01_t(1, psA, 1)
                phase01_t(2, psA, 7)

            # ---- S9: l1 GCN @1 + next-layer X63 + l2 GCN @7 -------------
            with (
                tc.tile_pool(name="psSC1b", bufs=1, space="PSUM") as psSC,
                tc.tile_pool(name="psGE1b", bufs=1, space="PSUM") as psGE,
                tc.tile_pool(name="eP1b", bufs=1) as eP,
                tc.tile_pool(name="wkP1b", bufs=1) as wkP,
            ):
                l1gcn(1, psSC, psGE, eP, wkP)
                x63_next(1, psSC, wkP, x63_b, x63_h)
                # l2 gcn@7 needs only the new X63 + its phase01 (done in
                # S7): run it here so it overlaps S11/S12's serial chains
                gcn_t(2, "l2t7", 7, x63_h[:], xtb_h[:, _tcol(7)],
                      xwones[:, 1024 * 7:1024 * 7 + 1024],
                      z_sb[:, _tcol(7)], xg_h[:, _tcol(7)],
                      psSC, psGE, None, eP, wkP, None, 3, 2)

            # ---- S11: l1 conv@3 + p-chain (halo landed) -----------------
            with (
                tc.tile_pool(name="psY1b", bufs=1, space="PSUM") as psY,
                tc.tile_pool(name="yP1b", bufs=1) as yP,
            ):
                mask_halo(halo_b)
                y_ps, orows = conv_taps(1, 3, psY, "y", 2)
                conv_finish(1, 3, y_ps, orows, yP, upd='xh')
                pchain_conv(1, yP, psY)

            # ---- S12: l2 phase01 for t=3 + p-chain ----------------------
            with tc.tile_pool(name="psA2b", bufs=1, space="PSUM") as psA:
                phase01_t(2, psA, 3)
                phase01_p(2, psA)

            # ---- S13/S14: l2 GCN (t=3,7,p) + conv@7 (virtual) -----------
            with (
                tc.tile_pool(name="psSC2", bufs=1, space="PSUM") as psSC,
                tc.tile_pool(name="psGE2", bufs=1, space="PSUM") as psGE,
                tc.tile_pool(name="eP2", bufs=1) as eP,
                tc.tile_pool(name="wkP2", bufs=1) as wkP,
            ):
                pend = gcn_t(2, "l2t3", 3, x63_h[:], xtb_h[:, _tcol(3)],
                             xwones[:, 1024 * 3:1024 * 3 + 1024],
                             z_sb[:, _tcol(3)], xg_h[:, _tcol(3)],
                             psSC, psGE, None, eP, wkP, None, 3, 2,
                             defer_combine=True)
                pend = gcn_t(2, "l2tp", 0, x63_h[:], xtbp_h[:], xwonesp[:],
                             zp[:], xgp_h[:],
                             psSC, psGE, None, eP, wkP, None, 3, 2,
                             pre=pend, defer_combine=True)
                pend()
                y_ps, orows = conv_taps(2, TLOC - 1, psSC, "sc", 3,
                                        use_virtual=True)
                nc.scalar.activation(_cs3(res[2][orows, :]),
                                     _ps3(y_ps[orows, :]), AF.Relu,
                                     bias=w_cb[orows, 2:3])

            # ===================== attention (all cores) ==================
            with (
                tc.tile_pool(name="psAT", bufs=1, space="PSUM") as psAT,
                tc.tile_pool(name="atP", bufs=1) as atP,
            ):
                S_t = []
                for m in range(L):
                    sm = atP.tile([128, N], f32, name=f"S{m}", tag="S",
                                  bufs=3)
                    nc.scalar.activation(
                        sm[ODD, :], res[m][ODD, :], AF.Tanh,
                        bias=attw[ODD, 1:2], scale=attw[ODD, 0:1])
                    S_t.append(sm)
                for c in range(NCH):
                    sz = CH_SZ[c]
                    co = CH_OFF[c]
                    s_ps = psAT.tile([128, 4], f32, name=f"sps{c}",
                                     tag="sps", bufs=2)
                    for m in range(L):
                        nc.tensor.matmul(
                            s_ps[0:sz, m:m + 1], S_t[m][ODD, co:co + sz],
                            attw[ODD, 2:3], start=True, stop=True,
                            tile_position=(64, 0))
                    e_s = atP.tile([128, 4], f32, name=f"es{c}", tag="es",
                                   bufs=2)
                    nc.scalar.activation(e_s[0:sz, 0:3], s_ps[0:sz, 0:3],
                                         AF.Exp)
                    den = atP.tile([128, 1], f32, name=f"den{c}", tag="den",
                                   bufs=2)
                    nc.vector.tensor_reduce(
                        den[0:sz, :], e_s[0:sz, 0:3], mybir.AxisListType.X,
                        OP.add)
                    rden = atP.tile([128, 1], f32, name=f"rden{c}",
                                    tag="rden", bufs=2)
                    nc.vector.reciprocal(rden[0:sz, :], den[0:sz, :])
                    aw = atP.tile([128, 4], f32, name=f"aw{c}", tag="aw",
                                  bufs=2)
                    nc.vector.tensor_scalar_mul(
                        aw[0:sz, 0:3], e_s[0:sz, 0:3], rden[0:sz, :])
                    acc = atP.tile([128, 64], f32, name=f"acc{c}",
                                   tag="acc", bufs=2)
                    for m in range(L):
                        rt_ps = psAT.tile([128, 64], f32, name=f"rt{c}{m}",
                                          tag="rt", bufs=2)
                        nc.tensor.transpose(
                            rt_ps[0:sz, :], res[m][ODD, co:co + sz],
                            ident[ODD, :], tile_position=(64, 0))
                        if m == 0:
                            nc.vector.tensor_scalar_mul(
                                acc[0:sz, :], rt_ps[0:sz, :],
                                aw[0:sz, m:m + 1])
                        else:
                            tmp = atP.tile([128, 64], f32, name=f"tmp{c}{m}",
                                           tag="tmp", bufs=2)
                            nc.vector.tensor_scalar_mul(
                                tmp[0:sz, :], rt_ps[0:sz, :],
                                aw[0:sz, m:m + 1])
                            nc.vector.tensor_add(
                                acc[0:sz, :], acc[0:sz, :], tmp[0:sz, :])
                    nc.sync.dma_start(out_t[co:co + sz, :], acc[0:sz, :])

        perm.release()

    _split_multi_waits(nc)
    return nc


# ---------------------------------------------------------------------------
def _prep_inputs(inputs):
    """Build the 8 per-core input maps from the full problem inputs."""
    X = np.ascontiguousarray(np.asarray(inputs["node_embeddings"], np.float32))
    B = np.asarray(inputs["B_weight"], np.float32)
    MTE = np.asarray(inputs["static_MTE"], np.float32)
    gcn_W = np.asarray(inputs["gcn_W"], np.float32)
    gcn_b = np.asarray(inputs["gcn_b"], np.float32)
    conv_W = np.asarray(inputs["conv_W"], np.float32)
    conv_b = np.asarray(inputs["conv_b"], np.float32)
    W_a = np.asarray(inputs["W_a"], np.float32)
    v = np.asarray(inputs["v"], np.float32)
    b_a = np.asarray(inputs["b_a"], np.float32)

    def dup(a):  # [64, k] -> [128, k]
        return np.ascontiguousarray(np.concatenate([a, a], axis=0))

    bdup = dup(B.astype(np.float16))
    gws = [dup(gcn_W[l].astype(np.float16)) for l in range(L)]
    cws = []
    for l in range(L):
        blocks = [np.ascontiguousarray(conv_W[l, :, :, k].T).astype(np.float16)
                  for k in range(KSZ)]
        cws.append(np.ascontiguousarray(np.concatenate(blocks, axis=1)))
    gbias = np.ascontiguousarray(gcn_b.T).astype(np.float32)        # [64, 3]
    cbias = dup(np.ascontiguousarray(conv_b.T).astype(np.float32))  # [128, 3]
    onesh = np.ones((128, 512), np.float16)
    ident = dup(np.eye(64, dtype=np.float32))                       # [128, 64]
    attp = np.zeros((64, 4), np.float32)
    attp[:, 0] = np.diagonal(W_a)
    attp[:, 1] = b_a
    attp[:, 2] = v[:, 0]
    attp = dup(attp)                                                # [128, 4]

    x63h = np.ascontiguousarray(X[T - 1].T).astype(np.float16)      # [64, N]

    in_maps = []
    for j in range(NCORES):
        Xl = X[TLOC * j:TLOC * (j + 1)]              # [8, N, D]
        xl = np.zeros((128, 4000), np.float32)
        for i in range(TLOC):
            rs = slice(0, 64) if i % 2 == 0 else slice(64, 128)
            xl[rs, 1000 * (i // 2):1000 * (i // 2) + 1000] = Xl[i].T
        xprev = (np.ascontiguousarray(X[TLOC * j - 1].T) if j > 0
                 else np.zeros((D, N), np.float32)).astype(np.float32)
        src = (j - 1) % NCORES
        hoff = np.zeros((64, 8), np.int32)
        for k in range(3):
            hoff[:, k] = (src * 3 + k) * 64 + np.arange(64)
        hmask = np.full((64, 1), 0.0 if j == 0 else 1.0, np.float32)
        in_maps.append({
            "mte": np.ascontiguousarray(
                MTE[TLOC * j:TLOC * (j + 1)] * MTE_SCALE).astype(
                    ml_dtypes.float8_e4m3),
            "x_loc": xl,
            "x16_loc": xl.astype(np.float16),
            "x63h_in": x63h,
            "xprev_in": np.ascontiguousarray(xprev),
            "bdup": bdup,
            **{f"gw{l}": gws[l] for l in range(L)},
            **{f"cw{l}": cws[l] for l in range(L)},
            "gbias": gbias,
            "cbias": cbias,
            "onesh_in": onesh,
            "hoff_in": hoff,
            "hmask_in": hmask,
            "ident_in": ident,
            "attp": attp,
        })
    return in_maps


_prog_cache = {}


def run_kernel(inputs, trace=False):
    alpha = float(np.asarray(inputs["alpha"]))
    if alpha not in _prog_cache:
        _prog_cache[alpha] = build_program(alpha)
    nc = _prog_cache[alpha]
    in_maps = _prep_inputs(inputs)
    res = run_bass_kernel_spmd(nc, in_maps, list(range(NCORES)), trace=trace)
    out = np.asarray(res.results[7]["out"], np.float32)
    return out, res


def kernel(**inputs) -> np.ndarray:
    out, _ = run_kernel(inputs, trace=False)
    return out
